# revision 2
# baseline (speedup 1.0000x reference)
"""Trainium2 Bass kernel for 2-layer GCN + mean-pool + GRU step + LN + linear.

Strategy (8 NeuronCores, SPMD single program, per-core data):
- Graph-aligned node sharding: core c owns graphs [256c, 256(c+1)) and their
  contiguous node range (batch is sorted), padded to NPC=13056 (102 tiles).
- Both GCN layers commute the weight matmul past aggregation; the per-edge
  norm dinv[src]*dinv[dst] is handled with NO per-edge multiply: the host
  prescales x by dinv (fp16 table), layer 1 writes relu(dinv^2 (z@W1)) so the
  h1 table is already dinv-scaled for layer 2, and dinv[dst] folds into the
  per-partition activation scale.
- Edges are packed DENSELY per (dst-tile, phase) — no per-lane slot rigidity:
- Gather (fp16, non-transpose, 4 phase windows via elem_step): edge-major
  output G [128 edges, chunk, 128 f]; one call covers up to 8 chunks
  (1024 idx cap).
- Aggregation z_T[f, lane] = sum_e G[e, f] * S[e, lane] as per-chunk PE
  matmuls accumulating in PSUM. S is a 0/1 selector built on-device per
  (group, phase) with a single DVE is_equal against an iota row, from a
  per-chunk lane-id table.
- Effective slot count ~= E/128 + rounding (~1900 vs 2996 for v2), cutting
  gather DMA, GPSIMD calls, and DVE work.
"""

import numpy as np
import os as _os

N_NODES = 100000
N_EDGES = 1600000
N_GRAPHS = 2048
D = 128
P = 128
EPS = 1e-5
NC = 8
GPC = N_GRAPHS // NC
NPC = 13056
TILES = NPC // P              # 102
TBL = NC * NPC                # 104448
NPH = 4
V3 = True
CHTILES = [int(v) for v in _os.environ.get(
    "CHTILES", "0,26,52,78,102").split(",")]
NCHUNK = len(CHTILES) - 1
CHUNK_BUDGET = int(_os.environ.get("CHUNK_BUDGET", "96"))  # chunks per group
CALL_CHUNKS = 8                                            # 1024-idx cap
GBUFS = int(_os.environ.get("GBUFS", "2"))

_CACHE = {}
_last_in_maps = None


def _prep(x, src, dst, batch):
    deg = np.bincount(dst, minlength=N_NODES).astype(np.float64) + 1.0
    dinv = 1.0 / np.sqrt(deg)

    node_start = np.searchsorted(batch, np.arange(0, N_GRAPHS + 1, GPC))
    core_of = np.searchsorted(node_start, np.arange(N_NODES), side="right") - 1

    a_src = np.concatenate([src, np.arange(N_NODES)])
    a_dst = np.concatenate([dst, np.arange(N_NODES)])

    # natural positions; global table layout is CHUNK-major so each of the
    # 4 pipelined AllGathers lands contiguously: chunk q holds tile range
    # CHTILES[q]..CHTILES[q+1] of every core.
    pos = np.empty(N_NODES, np.int64)
    for c in range(NC):
        lo, hi = node_start[c], node_start[c + 1]
        nloc = hi - lo
        assert nloc <= NPC - NPH
        pos[lo:hi] = np.arange(nloc)
    chunk_rows = [CHTILES[q + 1] * P - CHTILES[q] * P for q in range(NCHUNK)]
    chunk_base = np.concatenate([[0], np.cumsum([NC * r for r in chunk_rows])])
    start_q = np.array([CHTILES[q] * P for q in range(NCHUNK)])

    def to_gpos(core, p):
        q = np.searchsorted(start_q, p, side="right") - 1
        return (chunk_base[q] + core * np.array(chunk_rows)[q]
                + (p - start_q[q]))

    gpos = to_gpos(core_of, pos)
    phase_n = gpos % NPH        # node phase (as source) == pos % 4
    assert np.all(phase_n == pos % NPH)
    tile_of = pos // P
    lane_of = pos % P

    # per-(core, tile, phase) edge counts -> shared chunk schedule
    e_core = core_of[a_dst]
    t_e = tile_of[a_dst]
    ph_e = phase_n[a_src]
    cnt = np.zeros((NC, TILES, NPH), np.int64)
    np.add.at(cnt, (e_core, t_e, ph_e), 1)
    nchunks = -(-cnt.max(axis=0) // P)          # [TILES, NPH] shared
    chtot = int(nchunks.sum())

    # groups of consecutive tiles bounded by chunk budget
    ch_t = nchunks.sum(axis=1)
    groups = []
    t0 = 0
    while t0 < TILES:
        t1 = t0 + 1
        s = ch_t[t0]
        while t1 < TILES and s + ch_t[t1] <= CHUNK_BUDGET and (t1 - t0) < 8:
            s += ch_t[t1]
            t1 += 1
        groups.append((t0, t1))
        t0 = t1

    # chunk offsets: global order = (group, phase, tile)
    NGRP = len(groups)
    chunkbase = np.zeros((NGRP, NPH), np.int64)   # global chunk base of region
    regionch = np.zeros((NGRP, NPH), np.int64)    # chunks per region
    toff_rel = np.zeros((TILES, NPH), np.int64)   # chunk offset of tile in rgn
    pbase = np.zeros((NGRP, NPH), np.int64)       # chunk offset in g buffer
    grp_of = np.zeros(TILES, np.int64)
    gb = 0
    for gi, (g0, g1) in enumerate(groups):
        grp_of[g0:g1] = gi
        off = 0
        for ph in range(NPH):
            chunkbase[gi, ph] = gb + off
            pbase[gi, ph] = off
            for t in range(g0, g1):
                toff_rel[t, ph] = int(nchunks[g0:t, ph].sum())
            regionch[gi, ph] = int(nchunks[g0:g1, ph].sum())
            off += regionch[gi, ph]
        gb += off
    assert gb == chtot
    gmax = max(int(ch_t[g0:g1].sum()) for g0, g1 in groups)
    rmax = int(regionch.max())

    # per-core tables
    cntg = np.maximum(np.bincount(batch, minlength=N_GRAPHS), 1)
    idx_all, lane_all, spool_all, d1_all, d2_all = [], [], [], [], []
    for c in range(NC):
        m = e_core == c
        eu, ev = a_src[m], a_dst[m]
        te, pe_, le = t_e[m], ph_e[m], lane_of[ev]
        # j = ordinal within (tile, phase)
        key = te * NPH + pe_
        order = np.argsort(key, kind="stable")
        ks = key[order]
        first = np.r_[True, ks[1:] != ks[:-1]]
        sid = np.where(first, np.arange(ks.size), 0)
        j_o = np.arange(ks.size) - np.maximum.accumulate(sid)
        j = np.empty(ks.size, np.int64)
        j[order] = j_o
        assert np.all(j < nchunks[te, pe_] * P)

        zrow = to_gpos(np.full(NPH, c), (NPC - NPH) + np.arange(NPH)) // NPH
        fill = np.empty(8 * chtot, np.int16)
        for gi in range(NGRP):
            for ph in range(NPH):
                cb, nr = chunkbase[gi, ph], regionch[gi, ph]
                fill[8 * cb:8 * (cb + nr)] = zrow[ph]
        idx16 = np.tile(fill, (16, 1)).astype(np.int16)

        i_r = toff_rel[te, pe_] * P + j          # region-local edge ordinal
        col = 8 * chunkbase[grp_of[te], pe_] + i_r // 16
        row = i_r % 16
        idx16[row, col] = (gpos[eu] // NPH).astype(np.int16)
        idx_all.append(np.tile(idx16, (8, 1)))

        laneid = np.zeros((P, chtot), np.float16)
        gchunk = chunkbase[grp_of[te], pe_] + i_r // P
        laneid[i_r % P, gchunk] = le.astype(np.float16)
        lane_all.append(laneid)

        lo, hi = node_start[c], node_start[c + 1]
        d1 = np.zeros((P, TILES), np.float32)
        d2 = np.zeros((P, TILES), np.float32)
        d1[pos[lo:hi] % P, pos[lo:hi] // P] = (dinv[lo:hi] ** 2).astype(np.float32)
        d2[pos[lo:hi] % P, pos[lo:hi] // P] = dinv[lo:hi].astype(np.float32)
        d1_all.append(d1)
        d2_all.append(d2)

        spool = np.zeros((NPC, GPC), np.float32)
        gl = batch[lo:hi] - c * GPC
        spool[pos[lo:hi], gl] = (1.0 / cntg[batch[lo:hi]]).astype(np.float32)
        spool_all.append(spool)

    x_g = np.zeros((TBL, D), np.float16)
    x_g[gpos] = (x.astype(np.float64) * dinv[:, None]).astype(np.float16)

    iota = np.tile(np.arange(P, dtype=np.float16), (P, 1))

    return dict(
        x_g=x_g, idx=idx_all, laneid=lane_all, d1=d1_all, d2=d2_all,
        spool=spool_all, iota=iota, nchunks=nchunks, groups=groups,
        chunkbase=chunkbase, regionch=regionch, toff_rel=toff_rel,
        pbase=pbase, chtot=chtot, gmax=gmax, rmax=rmax,
    )


def _build(nchunks, groups, chunkbase, regionch, toff_rel, pbase, chtot,
           gmax, rmax, zero_b1, zero_b2):
    import concourse.bacc as bacc
    import concourse.mybir as mybir
    import concourse.tile as tile
    from concourse import library_config

    f32 = mybir.dt.float32
    fp16 = mybir.dt.float16
    Act = mybir.ActivationFunctionType
    Alu = mybir.AluOpType

    nc = bacc.Bacc("TRN2", target_bir_lowering=False, debug=False,
                   num_devices=NC, num_swdge_queues=4)

    x_g = nc.dram_tensor("x_g", [TBL, D], fp16, kind="ExternalInput")
    idx_in = nc.dram_tensor("idx", [P, 8 * chtot], mybir.dt.int16,
                            kind="ExternalInput")
    lane_in = nc.dram_tensor("laneid", [P, chtot], fp16, kind="ExternalInput")
    iota_in = nc.dram_tensor("iota", [P, P], fp16, kind="ExternalInput")
    d1_in = nc.dram_tensor("d1", [P, TILES], f32, kind="ExternalInput")
    d2_in = nc.dram_tensor("d2", [P, TILES], f32, kind="ExternalInput")
    spool_in = nc.dram_tensor("spool", [NPC, GPC], f32, kind="ExternalInput")
    w1_in = nc.dram_tensor("w1", [D, D], f32, kind="ExternalInput")
    w2_in = nc.dram_tensor("w2", [D, D], f32, kind="ExternalInput")
    wih_in = nc.dram_tensor("wih", [D, 3 * D], f32, kind="ExternalInput")
    bias_rz_in = nc.dram_tensor("bias_rz", [P, 2], f32, kind="ExternalInput")
    bias_n_in = nc.dram_tensor("bias_n", [P, 2], f32, kind="ExternalInput")
    wlin_in = nc.dram_tensor("wlin", [D, 1], f32, kind="ExternalInput")
    blin_in = nc.dram_tensor("blin", [1, 1], f32, kind="ExternalInput")
    b1_in = nc.dram_tensor("b1b", [P, D], f32, kind="ExternalInput")
    b2_in = nc.dram_tensor("b2b", [P, D], f32, kind="ExternalInput")
    out = nc.dram_tensor("out", [1, GPC], f32, kind="ExternalOutput")

    ag_in = nc.dram_tensor("ag_in", [NPC, D], fp16, kind="Internal")
    h1g = nc.dram_tensor("h1g", [TBL, D], fp16, kind="Internal",
                         addr_space="Shared")

    nc.gpsimd.load_library(library_config.mlp)

    with tile.TileContext(nc) as tc:
        with (
            tc.tile_pool(name="io", bufs=1) as io,
            tc.tile_pool(name="gp", bufs=GBUFS) as gp,
            tc.tile_pool(name="sl", bufs=2) as sl,
            tc.tile_pool(name="wk", bufs=3) as wk,
            tc.tile_pool(name="sp", bufs=2) as sp,
            tc.tile_pool(name="ps_t", bufs=2, space="PSUM") as ps_t,
            tc.tile_pool(name="ps_m", bufs=2, space="PSUM") as ps_m,
            tc.tile_pool(name="ps_pool", bufs=1, space="PSUM") as ps_pool,
            tc.tile_pool(name="ps_h", bufs=2, space="PSUM") as ps_h,
        ):
            idx_t = io.tile([P, 8 * chtot], mybir.dt.int16)
            lane_t = io.tile([P, chtot], fp16)
            iota_t = io.tile([P, P], fp16)
            d1_t = io.tile([P, TILES], f32)
            d2_t = io.tile([P, TILES], f32)
            w1_t = io.tile([D, D], f32)
            w2_t = io.tile([D, D], f32)
            nc.sync.dma_start(out=idx_t[:], in_=idx_in[:])
            nc.sync.dma_start(out=lane_t[:], in_=lane_in[:])
            nc.sync.dma_start(out=iota_t[:], in_=iota_in[:])
            nc.sync.dma_start(out=d1_t[:], in_=d1_in[:])
            nc.sync.dma_start(out=d2_t[:], in_=d2_in[:])
            nc.sync.dma_start(out=w1_t[:], in_=w1_in[:])
            nc.sync.dma_start(out=w2_t[:], in_=w2_in[:])
            b1_t = io.tile([P, D], f32)
            b2_t = io.tile([P, D], f32)
            if not zero_b1:
                nc.sync.dma_start(out=b1_t[:], in_=b1_in[:])
            if not zero_b2:
                nc.sync.dma_start(out=b2_t[:], in_=b2_in[:])

            def _emit_chunk_ag(q):
                r0, r1 = CHTILES[q] * P, CHTILES[q + 1] * P
                rows = r1 - r0
                base = sum(NC * (CHTILES[i + 1] - CHTILES[i]) * P
                           for i in range(q))
                nc.gpsimd.collective_compute(
                    "AllGather", Alu.bypass, replica_groups=[list(range(NC))],
                    ins=[ag_in[r0:r1, :]],
                    outs=[h1g[base:base + NC * rows, :]],
                )

            def layer(table, w_t, is_l1):
                win = table[:].rearrange("(r four) d -> r (four d)", four=4)
                pool_ps = None
                if not is_l1:
                    pool_ps = ps_pool.tile([P, GPC], f32, space="PSUM")
                for gi, (g0, g1) in enumerate(groups):
                    g = gp.tile([P, gmax * P], fp16, tag="g")
                    ss = []
                    for ph in range(NPH):
                        nr = int(regionch[gi, ph])
                        cb = int(chunkbase[gi, ph])
                        pb = int(pbase[gi, ph])
                        c0 = 0
                        while c0 < nr:
                            ck = min(nr - c0, CALL_CHUNKS)
                            nidx = ck * P
                            nc.gpsimd.dma_gather(
                                out_ap=g[:, (pb + c0) * P:
                                         (pb + c0 + ck) * P].rearrange(
                                    "p (k f) -> p k f", k=ck),
                                in_ap=win[:, ph * D:(ph + 1) * D],
                                idxs_ap=idx_t[:, 8 * (cb + c0):
                                              8 * (cb + c0 + ck)],
                                num_idxs=nidx,
                                num_idxs_reg=nidx,
                                elem_size=D,
                                elem_step=4 * D,
                                transpose=False,
                                queue_num=ph,
                            )
                            c0 += ck
                        # selector for the whole region
                        if nr > 0:
                            s_t = sl.tile([P, rmax * P], fp16, tag=f"s{ph}")
                            nc.vector.tensor_tensor(
                                out=s_t[:, :nr * P].rearrange(
                                    "p (c l) -> p c l", c=nr),
                                in0=iota_t[:].unsqueeze(1)
                                .to_broadcast([P, nr, P]),
                                in1=lane_t[:, cb:cb + nr].unsqueeze(2)
                                .to_broadcast([P, nr, P]),
                                op=Alu.is_equal)
                            ss.append(s_t)
                        else:
                            ss.append(None)
                    for t in range(g0, g1):
                        nch_t = int(nchunks[t].sum())
                        if nch_t == 0:
                            # empty tile: rows must be exact zeros (the
                            # gather's zero-row padding points here)
                            if is_l1:
                                h0 = wk.tile([P, D], fp16, tag="h16")
                                nc.vector.memset(h0[:], 0.0)
                                nc.sync.dma_start(
                                    out=ag_in[t * P:(t + 1) * P, :],
                                    in_=h0[:])
                                if t + 1 in CHTILES:
                                    _emit_chunk_ag(CHTILES.index(t + 1) - 1)
                            else:
                                h0 = wk.tile([P, D], f32, tag="h2")
                                nc.vector.memset(h0[:], 0.0)
                                spt = sp.tile([P, GPC], f32, tag="sp")
                                nc.sync.dma_start(
                                    out=spt[:],
                                    in_=spool_in[t * P:(t + 1) * P, :])
                                nc.tensor.matmul(out=pool_ps[:], lhsT=h0[:],
                                                 rhs=spt[:], start=(t == 0),
                                                 stop=(t == TILES - 1))
                            continue
                        zt_ps = ps_t.tile([P, P], f32, space="PSUM", tag="zt")
                        done = 0
                        for ph in range(NPH):
                            nct = int(nchunks[t, ph])
                            if nct == 0:
                                continue
                            a = int(pbase[gi, ph] + toff_rel[t, ph])
                            sa = int(toff_rel[t, ph])
                            for j in range(nct):
                                nc.tensor.matmul(
                                    out=zt_ps[:],
                                    lhsT=g[:, (a + j) * P:(a + j + 1) * P],
                                    rhs=ss[ph][:, (sa + j) * P:
                                               (sa + j + 1) * P],
                                    start=(done == 0),
                                    stop=(done == nch_t - 1))
                                done += 1
                        zt = wk.tile([P, P], f32, tag="ztsb")
                        nc.scalar.copy(out=zt[:], in_=zt_ps[:])
                        h_ps = ps_m.tile([P, D], f32, space="PSUM", tag="h")
                        nc.tensor.matmul(out=h_ps[:], lhsT=zt[:], rhs=w_t[:],
                                         start=True, stop=True)
                        if is_l1:
                            h16 = wk.tile([P, D], fp16, tag="h16")
                            sc = d1_t[:, t:t + 1]
                            if zero_b1:
                                nc.scalar.activation(out=h16[:], in_=h_ps[:],
                                                     func=Act.Relu, scale=sc)
                            else:
                                hb = wk.tile([P, D], f32, tag="hb")
                                bb = wk.tile([P, D], f32, tag="bb")
                                nc.vector.tensor_scalar_mul(hb[:], h_ps[:], sc)
                                nc.vector.tensor_scalar_mul(
                                    bb[:], b1_t[:], d2_t[:, t:t + 1])
                                nc.vector.tensor_tensor(out=hb[:], in0=hb[:],
                                                        in1=bb[:], op=Alu.add)
                                nc.scalar.activation(out=h16[:], in_=hb[:],
                                                     func=Act.Relu)
                            nc.sync.dma_start(out=ag_in[t * P:(t + 1) * P, :],
                                              in_=h16[:])
                            if t + 1 in CHTILES:
                                _emit_chunk_ag(CHTILES.index(t + 1) - 1)
                        else:
                            sc = d2_t[:, t:t + 1]
                            h2 = wk.tile([P, D], f32, tag="h2")
                            if zero_b2:
                                nc.scalar.mul(out=h2[:], in_=h_ps[:], mul=sc)
                            else:
                                nc.vector.tensor_scalar_mul(h2[:], h_ps[:], sc)
                                nc.vector.tensor_tensor(out=h2[:], in0=h2[:],
                                                        in1=b2_t[:],
                                                        op=Alu.add)
                            spt = sp.tile([P, GPC], f32, tag="sp")
                            nc.sync.dma_start(
                                out=spt[:], in_=spool_in[t * P:(t + 1) * P, :])
                            nc.tensor.matmul(out=pool_ps[:], lhsT=h2[:],
                                             rhs=spt[:], start=(t == 0),
                                             stop=(t == TILES - 1))
                return pool_ps

            layer(x_g, w1_t, True)
            pool_ps = layer(h1g, w2_t, False)

            # ---- head: gT = pooled mean [128 h, 256 g] ----
            wih_t = io.tile([D, 3 * D], f32)
            brz_t = io.tile([P, 2], f32)
            bn_t = io.tile([P, 2], f32)
            wlin_t = io.tile([D, 1], f32)
            blin_t = io.tile([1, 1], f32)
            ones_m = io.tile([P, 1], f32)
            eps_t = io.tile([1, 1], f32)
            ones_r = io.tile([1, P], f32)
            nc.sync.dma_start(out=wih_t[:], in_=wih_in[:])
            nc.sync.dma_start(out=brz_t[:], in_=bias_rz_in[:])
            nc.sync.dma_start(out=bn_t[:], in_=bias_n_in[:])
            nc.sync.dma_start(out=wlin_t[:], in_=wlin_in[:])
            nc.sync.dma_start(out=blin_t[:], in_=blin_in[:])
            nc.vector.memset(ones_m[:], 1.0 / P)
            nc.vector.memset(eps_t[:], EPS)
            nc.vector.memset(ones_r[:], 1.0)

            hd = wk.tile([P, GPC], f32, tag="hd")
            nc.scalar.copy(out=hd[:], in_=pool_ps[:])

            def gate_mm(sl_):
                ps = ps_h.tile([P, GPC], f32, space="PSUM", tag="hps")
                nc.tensor.matmul(out=ps[:],
                                 lhsT=wih_t[:, sl_ * D:(sl_ + 1) * D],
                                 rhs=hd[:], start=True, stop=True)
                return ps

            r = wk.tile([P, GPC], f32, tag="r")
            nc.scalar.activation(out=r[:], in_=gate_mm(0)[:], func=Act.Sigmoid,
                                 bias=brz_t[:, 0:1], scale=1.0)
            zz = wk.tile([P, GPC], f32, tag="zz")
            nc.scalar.activation(out=zz[:], in_=gate_mm(1)[:],
                                 func=Act.Sigmoid, bias=brz_t[:, 1:2],
                                 scale=1.0)
            nps = gate_mm(2)
            rb = wk.tile([P, GPC], f32, tag="rb")
            nc.vector.tensor_scalar_mul(rb[:], r[:], bn_t[:, 1:2])
            t1 = wk.tile([P, GPC], f32, tag="t1")
            nc.vector.tensor_tensor(out=t1[:], in0=nps[:], in1=rb[:],
                                    op=Alu.add)
            n_t = wk.tile([P, GPC], f32, tag="nt")
            nc.scalar.activation(out=n_t[:], in_=t1[:], func=Act.Tanh,
                                 bias=bn_t[:, 0:1], scale=1.0)
            zn = wk.tile([P, GPC], f32, tag="zn")
            nc.vector.tensor_tensor(out=zn[:], in0=zz[:], in1=n_t[:],
                                    op=Alu.mult)
            hr = wk.tile([P, GPC], f32, tag="hr")
            nc.vector.tensor_tensor(out=hr[:], in0=n_t[:], in1=zn[:],
                                    op=Alu.subtract)
            nc.scalar.activation(out=hr[:], in_=hr[:], func=Act.Relu)

            mu_ps = ps_h.tile([1, GPC], f32, space="PSUM", tag="hps")
            nc.tensor.matmul(out=mu_ps[:], lhsT=ones_m[:], rhs=hr[:],
                             start=True, stop=True)
            mu = wk.tile([1, GPC], f32, tag="mu")
            nc.scalar.copy(out=mu[:], in_=mu_ps[:])
            mub_ps = ps_h.tile([P, GPC], f32, space="PSUM", tag="hps")
            nc.tensor.matmul(out=mub_ps[:], lhsT=ones_r[:], rhs=mu[:],
                             start=True, stop=True)
            dmu = wk.tile([P, GPC], f32, tag="dmu")
            nc.vector.tensor_tensor(out=dmu[:], in0=hr[:], in1=mub_ps[:],
                                    op=Alu.subtract)
            d2s = wk.tile([P, GPC], f32, tag="d2s")
            nc.scalar.activation(out=d2s[:], in_=dmu[:], func=Act.Square)
            var_ps = ps_h.tile([1, GPC], f32, space="PSUM", tag="hps")
            nc.tensor.matmul(out=var_ps[:], lhsT=ones_m[:], rhs=d2s[:],
                             start=True, stop=True)
            std = wk.tile([1, GPC], f32, tag="std")
            nc.scalar.activation(out=std[:], in_=var_ps[:], func=Act.Sqrt,
                                 bias=eps_t[:, 0:1])
            rstd = wk.tile([1, GPC], f32, tag="rstd")
            nc.vector.reciprocal(rstd[:], std[:])
            rsb_ps = ps_h.tile([P, GPC], f32, space="PSUM", tag="hps")
            nc.tensor.matmul(out=rsb_ps[:], lhsT=ones_r[:], rhs=rstd[:],
                             start=True, stop=True)
            gn = wk.tile([P, GPC], f32, tag="gn")
            nc.vector.tensor_tensor(out=gn[:], in0=dmu[:], in1=rsb_ps[:],
                                    op=Alu.mult)
            o_ps = ps_h.tile([1, GPC], f32, space="PSUM", tag="hps")
            nc.tensor.matmul(out=o_ps[:], lhsT=wlin_t[:], rhs=gn[:],
                             start=True, stop=True)
            o_sb = wk.tile([1, GPC], f32, tag="o")
            nc.scalar.activation(out=o_sb[:], in_=o_ps[:], func=Act.Identity,
                                 bias=blin_t[:, 0:1], scale=1.0)
            nc.sync.dma_start(out=out[:], in_=o_sb[:])

    nc.compile()
    return nc


def kernel(**inputs):
    x = np.ascontiguousarray(np.asarray(inputs["x"], dtype=np.float32))
    ei = np.asarray(inputs["edge_index"]).astype(np.int64)
    batch = np.asarray(inputs["batch"]).astype(np.int64)
    W1 = np.asarray(inputs["W1"], np.float32)
    b1 = np.asarray(inputs["b1"], np.float32)
    W2 = np.asarray(inputs["W2"], np.float32)
    b2 = np.asarray(inputs["b2"], np.float32)
    W_ih = np.asarray(inputs["W_ih"], np.float32)
    b_ih = np.asarray(inputs["b_ih"], np.float32)
    b_hh = np.asarray(inputs["b_hh"], np.float32)
    W_lin = np.asarray(inputs["W_lin"], np.float32)
    b_lin = np.asarray(inputs["b_lin"], np.float32)

    prep = _prep(x, ei[0], ei[1], batch)

    zero_b1 = not np.any(b1)
    zero_b2 = not np.any(b2)
    key = (prep["chtot"], zero_b1, zero_b2)
    if key not in _CACHE:
        _CACHE[key] = _build(prep["nchunks"], prep["groups"],
                             prep["chunkbase"], prep["regionch"],
                             prep["toff_rel"], prep["pbase"], prep["chtot"],
                             prep["gmax"], prep["rmax"], zero_b1, zero_b2)
    nc = _CACHE[key]

    wih = np.concatenate([W_ih[i * D:(i + 1) * D, :].T for i in range(3)],
                         axis=1).astype(np.float32)
    bias_rz = np.stack([b_ih[0:D] + b_hh[0:D], b_ih[D:2 * D] + b_hh[D:2 * D]],
                       axis=1).astype(np.float32)
    bias_n = np.stack([b_ih[2 * D:], b_hh[2 * D:]], axis=1).astype(np.float32)
    b1b = np.tile(b1[None, :], (P, 1)).astype(np.float32)
    b2b = np.tile(b2[None, :], (P, 1)).astype(np.float32)

    in_maps = []
    for c in range(NC):
        in_maps.append({
            "x_g": prep["x_g"],
            "idx": prep["idx"][c],
            "laneid": prep["laneid"][c],
            "iota": prep["iota"],
            "d1": prep["d1"][c],
            "d2": prep["d2"][c],
            "spool": prep["spool"][c],
            "w1": W1, "w2": W2,
            "wih": wih, "bias_rz": bias_rz, "bias_n": bias_n,
            "wlin": W_lin.T.astype(np.float32).reshape(D, 1),
            "blin": b_lin.reshape(1, 1).astype(np.float32),
            "b1b": b1b, "b2b": b2b,
        })

    global _last_in_maps
    _last_in_maps = in_maps
    from concourse.bass_utils import run_bass_kernel_spmd
    res = run_bass_kernel_spmd(nc, in_maps, core_ids=list(range(NC)))
    out = np.concatenate([res.results[c]["out"][0] for c in range(NC)])
    return out.reshape(N_GRAPHS, 1).astype(np.float32)
